# revision 23
# baseline (speedup 1.0000x reference)
"""CrossAttention kernel for 8 Trainium2 NeuronCores.

Sharding: batch (4) x query-half (2) -> 8 cores. Each core computes the
full 8-head attention for 1024 queries of one batch against that batch's
full 2048-key context, plus the output projection for those queries.
Outputs per core are disjoint slices of `out` and `attn`, so no cross-core
reduction is needed.

Device math runs in fp16 (fp32 PSUM accumulation); attn probabilities are
written fp16 and upcast to fp32 on the host during the gather step.

Layout notes (per core):
  xT   [D=1024, M=1024]  host-pretransposed x slice  (contraction dim on partitions)
  ctxT [D=1024, J=2048]  host-pretransposed context
  qT = Wq^T x^T : [INNER=512, M]  via lhsT=Wq chunk, rhs=xT chunk
  kT = Wk^T c^T : [INNER, J]
  v  = c Wv     : [J, INNER]      via lhsT=ctxT chunk, rhs=Wv chunk
  sim[h] (M x J) in PSUM via lhsT=qT[h] (64 x 128), rhs=kT[h] (64 x 512);
  softmax: exp on ScalarE with accum_out giving row sums for free, then a
  per-partition reciprocal multiply on VectorE.  attn tile written to HBM,
  read back transposed (XBAR DMA transpose, fp16) as pT [J x M] for the
  PV matmul: oT[h] = v[h]^T pT.  Output projection contracts oT (with an
  appended ones-row chunk) against [Wo; bo] so the bias needs no broadcast.
"""

import numpy as np

B, N, J = 4, 2048, 2048
D = 1024
H, DH = 8, 64
INNER = H * DH          # 512
P = 128
M = N // 2              # 1024 queries per core
NCORES = 8
SCALE = DH ** -0.5

DC = D // P             # 8 contraction chunks for the projections
IC = INNER // P         # 4 inner-dim chunks
JC = J // P             # 16 key chunks
MC = M // P             # 8 query chunks
WOC = IC + 1            # Wo chunks + bias chunk

_CACHED_NC = None
_LAST_IN_MAPS = None


def build_program():
    import concourse.mybir as mybir
    import concourse.tile as tile
    from concourse import bacc
    from contextlib import ExitStack

    fp16 = mybir.dt.float16
    fp32 = mybir.dt.float32

    nc = bacc.Bacc()

    xT_d = nc.dram_tensor("xT", [D, M], fp16, kind="ExternalInput")
    ctxT_d = nc.dram_tensor("ctxT", [D, J], fp16, kind="ExternalInput")
    wq_d = nc.dram_tensor("wq", [D, INNER], fp16, kind="ExternalInput")
    wk_d = nc.dram_tensor("wk", [D, INNER], fp16, kind="ExternalInput")
    wv_d = nc.dram_tensor("wv", [D, INNER], fp16, kind="ExternalInput")
    wop_d = nc.dram_tensor("wop", [WOC * P, D], fp16, kind="ExternalInput")
    attn_d = nc.dram_tensor("attn", [H, M, J], fp16, kind="ExternalOutput")
    out_d = nc.dram_tensor("out", [M, D], fp32, kind="ExternalOutput")

    with tile.TileContext(nc) as tc, ExitStack() as ctx:
        xpool = ctx.enter_context(tc.tile_pool(name="xp", bufs=1))
        wpool = ctx.enter_context(tc.tile_pool(name="wp", bufs=1))
        kqv = ctx.enter_context(tc.tile_pool(name="kqv", bufs=1))
        ppool = ctx.enter_context(tc.tile_pool(name="pp", bufs=3))
        ptpool = ctx.enter_context(tc.tile_pool(name="ptp", bufs=12))
        opool = ctx.enter_context(tc.tile_pool(name="op", bufs=1))
        spool = ctx.enter_context(tc.tile_pool(name="sp", bufs=4))
        outpool = ctx.enter_context(tc.tile_pool(name="outp", bufs=2))
        psA = ctx.enter_context(tc.tile_pool(name="psA", bufs=2, space="PSUM"))
        psB = ctx.enter_context(tc.tile_pool(name="psB", bufs=1, space="PSUM"))
        psC = ctx.enter_context(tc.tile_pool(name="psC", bufs=2, space="PSUM"))

        # ---- input loads (split per contraction chunk for early compute start)
        xT_sb = xpool.tile([P, DC, M], fp16)
        ctxT_sb = xpool.tile([P, DC, J], fp16)
        xT_r = xT_d.rearrange("(dc p) m -> p dc m", p=P)
        ctxT_r = ctxT_d.rearrange("(dc p) j -> p dc j", p=P)
        wq_sb = wpool.tile([P, DC, INNER], fp16)
        wk_sb = wpool.tile([P, DC, INNER], fp16)
        wv_sb = wpool.tile([P, DC, INNER], fp16)
        wq_r = wq_d.rearrange("(dc p) i -> p dc i", p=P)
        wk_r = wk_d.rearrange("(dc p) i -> p dc i", p=P)
        wv_r = wv_d.rearrange("(dc p) i -> p dc i", p=P)
        # qT's inputs first so projections start ASAP, then kT's, then v's
        for dc in range(DC):
            nc.sync.dma_start(wq_sb[:, dc, :], wq_r[:, dc, :])
            nc.sync.dma_start(xT_sb[:, dc, :], xT_r[:, dc, :])
        for dc in range(DC):
            nc.sync.dma_start(wk_sb[:, dc, :], wk_r[:, dc, :])
            nc.sync.dma_start(ctxT_sb[:, dc, :], ctxT_r[:, dc, :])
        for dc in range(DC):
            nc.sync.dma_start(wv_sb[:, dc, :], wv_r[:, dc, :])
        wop_sb = wpool.tile([P, WOC, D], fp16)
        nc.sync.dma_start(wop_sb, wop_d.rearrange("(ic p) d -> p ic d", p=P))

        # ---- projections (emitted interleaved with attention, see below)
        qT_sb = kqv.tile([P, IC, M], fp16)
        kT_sb = kqv.tile([P, IC, J], fp16)
        v_sb = kqv.tile([P, JC, INNER], fp16)

        def proj_qk(ic):
            for mb in range(M // 512):
                ps = psC.tile([P, 512], fp32, tag="proj", name="ps")
                for dc in range(DC):
                    nc.tensor.matmul(
                        ps,
                        lhsT=wq_sb[:, dc, ic * P:(ic + 1) * P],
                        rhs=xT_sb[:, dc, mb * 512:(mb + 1) * 512],
                        start=(dc == 0),
                        stop=(dc == DC - 1),
                    )
                nc.vector.tensor_copy(qT_sb[:, ic, mb * 512:(mb + 1) * 512], ps)
            for jb in range(J // 512):
                ps = psC.tile([P, 512], fp32, tag="proj", name="ps")
                for dc in range(DC):
                    nc.tensor.matmul(
                        ps,
                        lhsT=wk_sb[:, dc, ic * P:(ic + 1) * P],
                        rhs=ctxT_sb[:, dc, jb * 512:(jb + 1) * 512],
                        start=(dc == 0),
                        stop=(dc == DC - 1),
                    )
                nc.vector.tensor_copy(kT_sb[:, ic, jb * 512:(jb + 1) * 512], ps)

        def proj_v(jc_lo, jc_hi):
            for jc in range(jc_lo, jc_hi):
                ps = psC.tile([P, 512], fp32, tag="proj", name="ps")
                for dc in range(DC):
                    nc.tensor.matmul(
                        ps,
                        lhsT=ctxT_sb[:, dc, jc * P:(jc + 1) * P],
                        rhs=wv_sb[:, dc, :],
                        start=(dc == 0),
                        stop=(dc == DC - 1),
                    )
                nc.vector.tensor_copy(v_sb[:, jc, :], ps)

        # oT holds all heads' attention outputs [INNER, M] plus a ones-row
        # chunk that multiplies the bias row of wop.
        oT_sb = opool.tile([P, WOC, M], fp16)
        nc.vector.memset(oT_sb[:, IC, :], 0.0)
        nc.gpsimd.memset(oT_sb[0:1, IC, :], 1.0)

        # ---- attention, software-pipelined at head granularity:
        #   A(h):  QK sim -> exp(+row sums) -> normalize -> attn write
        #   T(h):  16 XBAR transpose read-backs of attn[h] (SP prefetches
        #          these during A(h+1))
        #   B(h):  PV matmuls consuming the prefetched pT tiles
        # Emission order: T(h-1), A(h), B(h-1) — so PE alternates dense QK
        # and PV bursts and never waits on the DRAM round trip.
        def pass_a(h):
            hp = (h % 2) * DH
            ic = h // 2
            for mc in range(MC):
                p_sb = ppool.tile([P, J], fp16, tag="p", name="p_sb")
                sums = spool.tile([P, 2], fp32, tag="sums", name="sums")
                for grp in range(2):
                    qk = psA.tile([P, 1024], fp32, tag="qk", name="qk")
                    for sub in range(2):
                        jb = grp * 2 + sub
                        nc.tensor.matmul(
                            qk[:, sub * 512:(sub + 1) * 512],
                            lhsT=qT_sb[hp:hp + DH, ic, mc * P:(mc + 1) * P],
                            rhs=kT_sb[hp:hp + DH, ic, jb * 512:(jb + 1) * 512],
                            start=True,
                            stop=True,
                        )
                    nc.scalar.activation(
                        p_sb[:, grp * 1024:(grp + 1) * 1024],
                        qk,
                        mybir.ActivationFunctionType.Exp,
                        scale=SCALE,
                        accum_out=sums[:, grp:grp + 1],
                    )
                ssum = spool.tile([P, 1], fp32, tag="ssum", name="ssum")
                nc.vector.tensor_add(ssum, sums[:, 0:1], sums[:, 1:2])
                recip = spool.tile([P, 1], fp32, tag="recip", name="recip")
                nc.vector.reciprocal(recip, ssum)
                pn_sb = ppool.tile([P, J], fp16, tag="pn", name="pn_sb")
                nc.vector.tensor_scalar_mul(pn_sb, p_sb, recip)
                nc.gpsimd.dma_start(attn_d[h, mc * P:(mc + 1) * P, :], pn_sb)

        def emit_transposes(h):
            pts = []
            for jc in range(JC):
                pt = ptpool.tile([P, M], fp16, tag="pt", name="pt")
                nc.sync.dma_start_transpose(pt, attn_d[h, :, jc * P:(jc + 1) * P])
                pts.append(pt)
            return pts

        def pass_b(h, pts):
            hp = (h % 2) * DH
            ic = h // 2
            ovs = [
                psB.tile([DH, 512], fp32, tag=f"ov{mb}", name=f"ov{mb}")
                for mb in range(M // 512)
            ]
            for jc in range(JC):
                for mb in range(M // 512):
                    nc.tensor.matmul(
                        ovs[mb],
                        lhsT=v_sb[:, jc, h * DH:(h + 1) * DH],
                        rhs=pts[jc][:, mb * 512:(mb + 1) * 512],
                        start=(jc == 0),
                        stop=(jc == JC - 1),
                    )
            for mb in range(M // 512):
                nc.scalar.copy(
                    oT_sb[hp:hp + DH, ic, mb * 512:(mb + 1) * 512], ovs[mb]
                )

        proj_qk(0)
        pts = None
        for h in range(H):
            if h >= 1:
                pts = emit_transposes(h - 1)
            if h % 2 == 0 and h // 2 + 1 < IC:
                proj_qk(h // 2 + 1)
            if h == 0:
                proj_v(0, JC // 2)
            elif h == 1:
                proj_v(JC // 2, JC)
            pass_a(h)
            if h >= 1:
                pass_b(h - 1, pts)
        pts = emit_transposes(H - 1)
        pass_b(H - 1, pts)

        # ---- output projection (bias folded in via the ones-row chunk)
        for mc in range(MC):
            for db in range(D // 512):
                ps = psC.tile([P, 512], fp32, tag="proj")
                for icc in range(WOC):
                    nc.tensor.matmul(
                        ps,
                        lhsT=oT_sb[:, icc, mc * P:(mc + 1) * P],
                        rhs=wop_sb[:, icc, db * 512:(db + 1) * 512],
                        start=(icc == 0),
                        stop=(icc == WOC - 1),
                    )
                o_sb = outpool.tile([P, 512], fp32, tag="out")
                nc.scalar.copy(o_sb, ps)
                nc.gpsimd.dma_start(out_d[mc * P:(mc + 1) * P, db * 512:(db + 1) * 512], o_sb)

    nc.compile()
    return nc


def _numpy_reference(x, context, mask, Wq, Wk, Wv, Wo, bo):
    """Slow host fallback, used only if mask is not all-True."""
    b, n, _ = x.shape
    j = context.shape[1]
    q = (x @ Wq).reshape(b, n, H, DH).transpose(0, 2, 1, 3)
    k = (context @ Wk).reshape(b, j, H, DH).transpose(0, 2, 1, 3)
    v = (context @ Wv).reshape(b, j, H, DH).transpose(0, 2, 1, 3)
    sim = np.einsum("bhnd,bhjd->bhnj", q, k) * SCALE
    neg = -np.finfo(sim.dtype).max
    sim = np.where(mask[:, None, None, :], sim, neg)
    sim = sim - sim.max(axis=-1, keepdims=True)
    e = np.exp(sim)
    attn = e / e.sum(axis=-1, keepdims=True)
    out = np.einsum("bhnj,bhjd->bhnd", attn, v)
    out = out.transpose(0, 2, 1, 3).reshape(b, n, INNER)
    out = out @ Wo + bo
    return out, attn


def kernel(**inputs):
    global _CACHED_NC
    from concourse.bass_utils import run_bass_kernel_spmd

    x = np.asarray(inputs["x"], dtype=np.float32)
    context = np.asarray(inputs["context"], dtype=np.float32)
    mask = np.asarray(inputs["mask"])
    Wq = np.asarray(inputs["Wq"], dtype=np.float32)
    Wk = np.asarray(inputs["Wk"], dtype=np.float32)
    Wv = np.asarray(inputs["Wv"], dtype=np.float32)
    Wo = np.asarray(inputs["Wo"], dtype=np.float32)
    bo = np.asarray(inputs["bo"], dtype=np.float32)

    if not mask.all():
        return _numpy_reference(x, context, mask, Wq, Wk, Wv, Wo, bo)

    if _CACHED_NC is None:
        _CACHED_NC = build_program()
    nc = _CACHED_NC

    wop = np.zeros((WOC * P, D), dtype=np.float16)
    wop[:INNER] = Wo.astype(np.float16)
    wop[INNER] = bo.astype(np.float16)

    wq16 = Wq.astype(np.float16)
    wk16 = Wk.astype(np.float16)
    wv16 = Wv.astype(np.float16)
    ctxT = [np.ascontiguousarray(context[b].T.astype(np.float16)) for b in range(B)]

    in_maps = []
    for c in range(NCORES):
        b, half = divmod(c, 2)
        xT = np.ascontiguousarray(
            x[b, half * M:(half + 1) * M, :].T.astype(np.float16)
        )
        in_maps.append(
            {
                "xT": xT,
                "ctxT": ctxT[b],
                "wq": wq16,
                "wk": wk16,
                "wv": wv16,
                "wop": wop,
            }
        )

    global _LAST_IN_MAPS
    _LAST_IN_MAPS = in_maps
    res = run_bass_kernel_spmd(nc, in_maps, core_ids=list(range(NCORES)))

    out = np.empty((B, N, D), dtype=np.float32)
    attn = np.empty((B, H, N, J), dtype=np.float32)
    for c in range(NCORES):
        b, half = divmod(c, 2)
        r = res.results[c]
        out[b, half * M:(half + 1) * M, :] = r["out"]
        attn[b, :, half * M:(half + 1) * M, :] = r["attn"].astype(np.float32)
    return out, attn
